# revision 54
# baseline (speedup 1.0000x reference)
"""Trainium2 Bass kernel for nn_BaseConvPlus (dense_cnn).

Math: the reference computes
  1) kernel[b,c,:,:]  = global-mean of a depthwise 3x3 conv of x          -> [B,CIN,3,3]
  2) win  = einsum(kernel, w_in) + b_in ; wout = einsum(kernel, w_out)
  3) y[b] = conv2d(x[b], weight[b]) with weight[b,o,i] = win[b,i]*wout[b,o]

Identities that make this memory-bound:
  * mean(conv(x, k)) over HxW only needs the total sum, edge-row/col sums
    and corner pixels of each channel (zero 'SAME' padding) - no conv.
    The tap-selection matrix is folded into the host-side wk tables, so
    kernel[b,c,j] = sum_k wkH[c,j,k] * sums[b,c,k] with sums = the 9
    reduced quantities [T, RF, RL, CF, CL, c00, c0L, cL0, cLL].
  * weight[b] is rank-1 across (o, i): y[b,o] = wout[b,o] * z[b] with
    z[b] = sum_i conv2d(x[b,i], win[b,i]).  The conv runs in two PE
    stages, 5 image passes total:
      stage1 (K=128=(b,i), bf16): 3 row-shifted matmuls contract (i, ky),
        with the 12 live lhsT columns DUPLICATED so PSUM gets two copies
        of G[(b,kx), pixels] at partitions 0:12 and 12:24.
      evict: the two copies go to SBUF with the kx column shift baked
        into the destination offset (ACT takes group0, DVE group1).
      stage2 (K=128 zero-padded, bf16): matmul A reads the packed buffer
        at base+1 covering kx=0 (group0) and kx=1 (group1) in ONE pass;
        matmul B re-reads group1 at base+2 for kx=2.  y lands in PSUM
        with wout applied; evicted to bf16 and DMA'd out.

Input is uploaded as bf16 (host cast) halving the load-phase DMA; the
seed statistics are computed from the bf16 image with partial sums
spread across DVE / ACT(accum) / GpSimd so they hide under the DMA.
Junk matmuls on otherwise-idle PE keep the HAM clock warm during load.

Sharding: pure data parallel, 4 samples per core on 8 cores.
"""
import sys

sys.path.insert(0, "/opt/trn_rl_repo")

from contextlib import ExitStack

import ml_dtypes
import numpy as np

import concourse.bacc as bacc
import concourse.bass as bass
import concourse.mybir as mybir
import concourse.tile as tile
from concourse.bass_utils import run_bass_kernel_spmd

B, CIN, COUT, KS, H, W = 32, 32, 32, 3, 192, 192
NCORES = 8
BC = B // NCORES          # 4 samples per core
P = BC * CIN              # 128 partitions = (sample, channel)
HP = H + 2                # 194 rows (one zero row above and below)
WP = W + 2                # 194: G gets the side padding instead of x
NPIX = HP * W             # 37248: row-padded pixels, rows contiguous
CHUNKS = [16] * 10 + [8] * 2 + [4] * 4   # input chunk rows (tapered tail)
NCHUNK = len(CHUNKS)      # 16
R = 2                     # output rows per conv tile
NT = H // R               # 96 conv tiles
NPAIR = NT // 2           # 48 stage1/evict pairs
NSLOT = 20                # G ring slots
LAG = 5                   # stage2 runs this many pairs behind stage1
RB = 2                    # stage1 pairs per replica-DMA batch
GT = 8                    # conv tiles per output DMA (16 rows)
NG = NT // GT             # 12 output DMAs
F32 = mybir.dt.float32
BF16 = mybir.dt.bfloat16
AX = mybir.AxisListType
OP = mybir.AluOpType
ACTF = mybir.ActivationFunctionType


def build_program(nc: bass.Bass) -> None:
    x_d = nc.dram_tensor("x", [BC, CIN, H, W], BF16, kind="ExternalInput").ap()
    wkh_d = nc.dram_tensor("wkh", [P, 81], F32, kind="ExternalInput").ap()
    lwin_d = nc.dram_tensor("lwin", [P, P], BF16, kind="ExternalInput").ap()
    brep_d = nc.dram_tensor("brep", [P, 1], F32, kind="ExternalInput").ap()
    wo9_d = nc.dram_tensor("wo9", [P, 9 * P], BF16, kind="ExternalInput").ap()
    m12_d = nc.dram_tensor("m12", [P, 12], F32, kind="ExternalInput").ap()
    ma_d = nc.dram_tensor("ma", [96, 1], F32, kind="ExternalInput").ap()
    ident_d = nc.dram_tensor("ident", [P, P], F32, kind="ExternalInput").ap()
    y_d = nc.dram_tensor("y", [BC, COUT, H, W], BF16, kind="ExternalOutput").ap()

    xf = x_d.rearrange("b c h w -> (b c) (h w)")       # [128, 36864] bf16
    yf = y_d.rearrange("b o h w -> (b o) (h w)")       # [128, 36864] bf16

    with tile.TileContext(nc) as tc, ExitStack() as ctx:
        const = ctx.enter_context(tc.tile_pool(name="const", bufs=1))
        ypool = ctx.enter_context(tc.tile_pool(name="ysb", bufs=3))
        psum_g = ctx.enter_context(tc.tile_pool(name="psum_g", bufs=2, space="PSUM"))
        psum_y = ctx.enter_context(tc.tile_pool(name="psum_y", bufs=2, space="PSUM"))

        xpad = const.tile([P, NPIX], BF16)
        wkh = const.tile([P, 81], F32)
        lwin = const.tile([P, P], BF16)
        brep = const.tile([P, 1], F32)
        wo9 = const.tile([P, 9 * P], BF16)
        m12 = const.tile([P, 12], F32)
        ma = const.tile([96, 1], F32)
        ident = const.tile([P, P], F32)
        scr = const.tile([P, 16 + 3 * NCHUNK], F32)  # 0:T 1:CF 2:CL 3:RF 4:RL 5..8 corners, then partials
        t81 = const.tile([P, 81], F32)
        kern = const.tile([P, 9], F32)
        kernb = const.tile([P, 9], BF16)
        vout = const.tile([P, 96], F32)
        lky = const.tile([P, 3 * P], BF16)     # stage1 lhsT per ky (cols (b,kx) live)
        lkx = const.tile([P, P], BF16)         # stage2 lhsT (kx=g in window g*32)
        gbuf = const.tile([P, NSLOT * R * WP], BF16)
        trash = const.tile([P, 16 * W], BF16)  # ACT accum-reduce dummy out

        x3 = xpad[:].rearrange("p (r c) -> p r c", c=W)    # [128, 194, 192]
        g4 = gbuf[:].rearrange("p (s r c) -> p s r c", s=NSLOT, c=WP)

        # constants ride the gpsimd (SWDGE) queue, parallel to the input;
        # issue them before the big gpsimd memsets so they are not delayed
        nc.gpsimd.dma_start(out=wkh[:], in_=wkh_d)
        nc.gpsimd.dma_start(out=lwin[:], in_=lwin_d)
        nc.gpsimd.dma_start(out=brep[:], in_=brep_d)
        nc.gpsimd.dma_start(out=wo9[:], in_=wo9_d)
        nc.gpsimd.dma_start(out=m12[:], in_=m12_d)
        nc.gpsimd.dma_start(out=ma[:], in_=ma_d)
        nc.gpsimd.dma_start(out=ident[:], in_=ident_d)

        # zero the two padding rows; column padding lives in the G buffer.
        # big memsets ride the otherwise-idle gpsimd engine
        nc.vector.memset(x3[:, 0, :], 0.0)
        nc.vector.memset(x3[:, HP - 1, :], 0.0)
        nc.gpsimd.memset(gbuf[:], 0.0)
        # lhsT tables are zero-padded to the full 128x128 array
        nc.gpsimd.memset(lky[:], 0.0)
        nc.gpsimd.memset(lkx[:], 0.0)
        nc.gpsimd.memset(vout[:], 0.0)

        # bf16 input lands straight in the padded image; partial sums for
        # the seed stats run on whichever engine has slack (T-sum touches
        # every element, so it is split DVE/ACT to hide under the DMA;
        # gpsimd has no free-axis reduce).  Tapered tail chunks keep the
        # last chunk's T-sum off the critical path.
        h0 = 0
        for i, lr in enumerate(CHUNKS):
            slot = x3[:, h0 + 1:h0 + 1 + lr, :]
            flat = xpad[:, (h0 + 1) * W:(h0 + 1 + lr) * W]
            nc.sync.dma_start(out=flat, in_=xf[:, h0 * W:(h0 + lr) * W])
            on_dve = i % 2 == 0 if i < NCHUNK - 2 else i == NCHUNK - 1
            if on_dve:
                nc.vector.reduce_sum(out=scr[:, 16 + i:17 + i], in_=flat, axis=AX.X)
            else:
                nc.scalar.activation(
                    out=trash[:, 0:lr * W], in_=flat, func=ACTF.Copy,
                    accum_out=scr[:, 16 + i:17 + i])
            nc.vector.reduce_sum(
                out=scr[:, 16 + NCHUNK + i:17 + NCHUNK + i],
                in_=slot[:, :, 0], axis=AX.X)
            nc.vector.reduce_sum(
                out=scr[:, 16 + 2 * NCHUNK + i:17 + 2 * NCHUNK + i],
                in_=slot[:, :, W - 1], axis=AX.X)
            if i == 0:      # row-0 sum and top corners only need chunk 0
                nc.vector.reduce_sum(out=scr[:, 3:4], in_=slot[:, 0, :], axis=AX.X)
                nc.vector.tensor_copy(scr[:, 5:7], slot[:, 0, 0:W:W - 1])
            if i == NCHUNK - 1:  # last-row sum and bottom corners
                nc.vector.reduce_sum(out=scr[:, 4:5], in_=slot[:, lr - 1, :], axis=AX.X)
                nc.vector.tensor_copy(scr[:, 7:9], slot[:, lr - 1, 0:W:W - 1])
            # junk matmuls on late chunks keep the PE HAM clock warm so
            # stage1 opens at 2.4 GHz (PE is otherwise idle until then)
            if 8 <= i < 16:
                jnk = psum_y.tile([P, R * W], F32, tag="yps", name="jnk")
                nj = min(5, lr * W // 384)
                for w in range(nj):
                    nc.tensor.matmul(
                        jnk[:], lhsT=lwin[:],
                        rhs=xpad[:, (h0 + 1) * W + w * 384:(h0 + 1) * W + (w + 1) * 384],
                        start=(w == 0), stop=(w == nj - 1))
            h0 += lr

        # final sums: T/CF/CL in one grouped reduce
        nc.vector.reduce_sum(
            out=scr[:, 0:3],
            in_=scr[:, 16:16 + 3 * NCHUNK].rearrange("p (g i) -> p g i", g=3),
            axis=AX.X)

        # kernel[p, j] = sum_k wkH[p, j*9+k] * sums[p, k]
        sums9 = scr[:, 0:9].unsqueeze(1).broadcast_to([P, 9, 9])
        nc.vector.tensor_mul(t81[:].rearrange("p (j m) -> p j m", m=9), wkh[:].rearrange("p (j m) -> p j m", m=9), sums9)
        nc.vector.reduce_sum(
            out=kern[:], in_=t81[:].rearrange("p (j m) -> p j m", m=9), axis=AX.X)
        nc.vector.tensor_copy(kernb[:], kern[:])

        # win = blockdiag(w_in.T) @ kernel (+ b_in fused into the lky build)
        win_ps = psum_y.tile([P, 9], F32, tag="yps", name="win_ps")
        nc.tensor.matmul(win_ps[:], lhsT=lwin[:], rhs=kernb[:], start=True, stop=True)

        # stage1 weights: lky[(b,i), (b',kx)] = (win[b,i,3ky+kx]+b_in) d(b,b')
        m123 = m12[:].rearrange("p (b k) -> p b k", k=3)
        for ky in range(3):
            wv = win_ps[:, 3 * ky:3 * ky + 3].unsqueeze(1).broadcast_to([P, BC, 3])
            nc.vector.scalar_tensor_tensor(
                lky[:, ky * P:ky * P + 12].rearrange("p (b k) -> p b k", k=3),
                wv, brep[:], m123, op0=OP.add, op1=OP.mult)

        # wout / stage2-weight chain is deferred: stage1 only needs lky,
        # so it is issued after the first stage1 pairs (stage2 trails by
        # LAG pairs and the chain completes well before it starts)
        def build_stage2_weights():
            # wout[(b,o)] = sum_j blockdiag(w_out[:,:,j].T) @ kernel[:, j]
            wout_ps = psum_y.tile([P, 1], F32, tag="yps", name="wout_ps")
            for j in range(9):
                nc.tensor.matmul(
                    wout_ps[:], lhsT=wo9[:, j * P:(j + 1) * P],
                    rhs=kernb[:, j:j + 1], start=(j == 0), stop=(j == 8))
            # stage2 weights: w96[(32g+(b,kx)),(b',o)] = wout[b',o] d(b,b'),
            # then the kx mask keeps kx==g in window g (one matmul covers
            # all three taps since each window's G replica is col-shifted)
            for g in range(3):
                nc.vector.tensor_scalar_mul(
                    vout[:, 32 * g:32 * g + 12], m12[:], wout_ps[:, 0:1])
            w96_ps = psum_y.tile([96, P], F32, tag="yps", name="w96_ps")
            nc.tensor.transpose(w96_ps[:], vout[:], ident[:])
            nc.vector.tensor_scalar_mul(lkx[0:96, :], w96_ps[:], ma[:, 0:1])

        # conv: stage1 (G into PSUM 0:12), ACT evict at column offset 2,
        # two DMA replicas at offsets 1/0 into partition windows 32/64,
        # stage2 (one matmul covers all three kx taps), DVE y evict
        ysb_tiles = {}
        gps_cur = [None]
        yps_cur = [None]

        def stage1(t):
            h0 = t * R
            if t % 2 == 0:
                gps_cur[0] = psum_g.tile([P, 1024], F32, tag="gps", name="gps")
            out = gps_cur[0][:, (t % 2) * 512:(t % 2) * 512 + R * W]
            for ky in range(3):
                nc.tensor.matmul(
                    out,
                    lhsT=lky[:, ky * P:(ky + 1) * P],
                    rhs=x3[:, h0 + ky:h0 + ky + R, :],
                    start=(ky == 0), stop=(ky == 2))

        def evict_pair(p):
            # window g holds G at column offset 2-g so stage2's base-1
            # read of window g yields G[w+g-1] (kx = g).  The replicas are
            # FLAT shifted copies of the whole pair region: row-crossing
            # elements land in pad columns that are either never read or
            # copied from window0's always-zero columns 0/1.
            s = (2 * p) % NSLOT
            src = gps_cur[0][0:12, :].rearrange(
                "p (k b) -> p k b", k=2)[:, :, 0:R * W].rearrange(
                "p k (r w) -> p k r w", w=W)
            nc.scalar.copy(out=g4[0:12, s:s + 2, :, 2:2 + W], in_=src)
            # replica DMAs are batched RB pairs at a time (the DIRECT2D
            # issue cost is per dma_start); the drain pairs go singly so
            # the tail of the pipeline is not waiting on a batch boundary
            if p >= NPAIR - 6 or p % RB == RB - 1:
                rb = 1 if p >= NPAIR - 6 else RB
                base = ((2 * (p - rb + 1)) % NSLOT) * R * WP
                nb = 2 * rb * R * WP
                nc.scalar.dma_start(
                    out=gbuf[32:44, base:base + nb - 1],
                    in_=gbuf[0:12, base + 1:base + nb])
                nc.gpsimd.dma_start(
                    out=gbuf[64:76, base:base + nb - 2],
                    in_=gbuf[0:12, base + 2:base + nb])

        def stage2(t):
            g = t // GT
            if g not in ysb_tiles:
                ysb_tiles[g] = ypool.tile(
                    [P, GT * R * W], BF16, tag="ysb", name="ysb")
            ysb = ysb_tiles[g]
            if t % 2 == 0:
                yps_cur[0] = psum_y.tile([P, 1024], F32, tag="yps", name="yps")
            y_ps = yps_cur[0][:, (t % 2) * 512:(t % 2) * 512 + R * W]
            s = t % NSLOT
            nc.tensor.matmul(
                y_ps, lhsT=lkx[:], rhs=g4[:, s, :, 1:1 + W],
                start=True, stop=True)
            if t % 2 == 1:
                tt = t % GT
                ysrc = yps_cur[0][:].rearrange(
                    "p (k b) -> p k b", k=2)[:, :, 0:R * W]
                ydst = ysb[:, (tt - 1) * R * W:(tt + 1) * R * W].rearrange(
                    "p (k f) -> p k f", k=2)
                if t // 2 >= NPAIR - LAG and (t // 2) % 2 == 1:
                    nc.scalar.copy(out=ydst, in_=ysrc)
                else:
                    nc.vector.tensor_copy(ydst, ysrc)
                if g == NG - 1:
                    nc.sync.dma_start(
                        out=yf[:, (g * GT + tt - 1) * R * W:(g * GT + tt + 1) * R * W],
                        in_=ysb[:, (tt - 1) * R * W:(tt + 1) * R * W])
                elif tt == GT - 1:
                    nc.sync.dma_start(
                        out=yf[:, g * GT * R * W:(g + 1) * GT * R * W],
                        in_=ysb[:])
                    del ysb_tiles[g]

        # stage2 trails stage1 by LAG pairs so the PE queue never drains
        # waiting on the evict -> replica-DMA chain
        for p in range(NPAIR):
            stage1(2 * p)
            stage1(2 * p + 1)
            if p == 0:
                build_stage2_weights()
            if p >= LAG:
                stage2(2 * (p - LAG))
                stage2(2 * (p - LAG) + 1)
            evict_pair(p)
        for p in range(NPAIR - LAG, NPAIR):
            stage2(2 * p)
            stage2(2 * p + 1)


def host_tables(wk, w_in, b_in, w_out):
    # H matrix: sums vector [T,CF,CL,RF,RL,c00,c0L,cL0,cLL] -> S[m], m=(dy,dx)
    Hm = np.zeros((9, 9), np.float32)
    Hm[0, :] = 1.0
    for m in range(9):
        dy, dx = divmod(m, 3)
        if dy == 0:
            Hm[4, m] -= 1.0
        if dy == 2:
            Hm[3, m] -= 1.0
        if dx == 0:
            Hm[2, m] -= 1.0
        if dx == 2:
            Hm[1, m] -= 1.0
    Hm[8, 0] = Hm[7, 2] = Hm[6, 6] = Hm[5, 8] = 1.0
    wk9 = wk.reshape(CIN, 9, 9).astype(np.float32) / float(H * W)  # [c, j, m]
    wkh = np.einsum("cjm,km->cjk", wk9, Hm).reshape(CIN, 81)
    wkh = np.tile(wkh, (BC, 1))

    lwin = np.kron(np.eye(BC, dtype=np.float32), w_in.T.astype(np.float32))
    brep = np.tile(b_in.astype(np.float32), BC)[:, None]
    w9 = w_out.reshape(COUT, CIN, 9).astype(np.float32)
    wo9 = np.concatenate(
        [np.kron(np.eye(BC, dtype=np.float32), w9[:, :, j].T) for j in range(9)],
        axis=1)
    # m12[(b~,i), (b,kx)] = d(b~==b)
    m12 = np.repeat(np.eye(BC, dtype=np.float32), CIN, axis=0)
    m12 = np.repeat(m12, 3, axis=1)  # [128, 12]
    # row j of the stage2 table is (g, b, kx') with g=j//32, kx'=(j%32)%3
    # in the live windows [32g, 32g+12): keep kx'==g (window g's G replica
    # is column-shifted so its base-1 read is the kx=g tap)
    j = np.arange(96)
    live = (j % 32) < 12
    kxp = (j % 32) % 3
    ma = (live & (kxp == j // 32)).astype(np.float32)[:, None]
    ident = np.eye(P, dtype=np.float32)
    return {
        "wkh": np.ascontiguousarray(wkh, np.float32),
        "lwin": np.ascontiguousarray(lwin).astype(ml_dtypes.bfloat16),
        "brep": np.ascontiguousarray(brep, np.float32),
        "wo9": np.ascontiguousarray(wo9).astype(ml_dtypes.bfloat16),
        "m12": np.ascontiguousarray(m12, np.float32),
        "ma": np.ascontiguousarray(ma, np.float32),
        "ident": np.ascontiguousarray(ident, np.float32),
    }


_CACHE: dict = {}


def _get_program() -> bass.Bass:
    if "nc" not in _CACHE:
        nc = bacc.Bacc(
            trn_type="TRN2", target_bir_lowering=False, debug=False,
            num_devices=NCORES)
        build_program(nc)
        nc.compile()
        _CACHE["nc"] = nc
    return _CACHE["nc"]


def kernel(x, wk, w_in, b_in, w_out, _trace=False, _trace_kwargs=None):
    x = np.asarray(x).astype(ml_dtypes.bfloat16)
    tables = host_tables(np.asarray(wk), np.asarray(w_in), np.asarray(b_in),
                         np.asarray(w_out))
    nc = _get_program()
    in_maps = [
        {"x": np.ascontiguousarray(x[c * BC:(c + 1) * BC]), **tables}
        for c in range(NCORES)
    ]
    res = run_bass_kernel_spmd(
        nc, in_maps, core_ids=list(range(NCORES)),
        trace=_trace, **(_trace_kwargs or {}))
    y = np.concatenate(
        [np.asarray(res.results[c]["y"]).astype(np.float32)
         for c in range(NCORES)], axis=0)
    if _trace:
        return y, res
    return y


if __name__ == "__main__":
    rng = np.random.default_rng(0)
    inputs = {
        "x": rng.standard_normal((B, CIN, H, W), np.float32),
        "wk": rng.standard_normal((CIN * 9, 1, 3, 3)).astype(np.float32) * 0.05,
        "w_in": rng.standard_normal((CIN, CIN)).astype(np.float32) * 0.05,
        "b_in": rng.standard_normal((CIN,)).astype(np.float32) * 0.05,
        "w_out": rng.standard_normal((COUT, CIN, 3, 3)).astype(np.float32) * 0.05,
    }
    y = kernel(**inputs)
    print("y", y.shape, y.dtype, float(np.abs(y).max()))


# revision 55
# speedup vs baseline: 1.0369x; 1.0369x over previous
"""Trainium2 Bass kernel for nn_BaseConvPlus (dense_cnn).

Math: the reference computes
  1) kernel[b,c,:,:]  = global-mean of a depthwise 3x3 conv of x          -> [B,CIN,3,3]
  2) win  = einsum(kernel, w_in) + b_in ; wout = einsum(kernel, w_out)
  3) y[b] = conv2d(x[b], weight[b]) with weight[b,o,i] = win[b,i]*wout[b,o]

Identities that make this memory-bound:
  * mean(conv(x, k)) over HxW only needs the total sum, edge-row/col sums
    and corner pixels of each channel (zero 'SAME' padding) - no conv.
    The tap-selection matrix is folded into the host-side wk tables, so
    kernel[b,c,j] = sum_k wkH[c,j,k] * sums[b,c,k] with sums = the 9
    reduced quantities [T, RF, RL, CF, CL, c00, c0L, cL0, cLL].
  * weight[b] is rank-1 across (o, i): y[b,o] = wout[b,o] * z[b] with
    z[b] = sum_i conv2d(x[b,i], win[b,i]).  The conv runs in two PE
    stages, 5 image passes total:
      stage1 (K=128=(b,i), bf16): 3 row-shifted matmuls contract (i, ky),
        with the 12 live lhsT columns DUPLICATED so PSUM gets two copies
        of G[(b,kx), pixels] at partitions 0:12 and 12:24.
      evict: the two copies go to SBUF with the kx column shift baked
        into the destination offset (ACT takes group0, DVE group1).
      stage2 (K=128 zero-padded, bf16): matmul A reads the packed buffer
        at base+1 covering kx=0 (group0) and kx=1 (group1) in ONE pass;
        matmul B re-reads group1 at base+2 for kx=2.  y lands in PSUM
        with wout applied; evicted to bf16 and DMA'd out.

Input is uploaded as bf16 (host cast) halving the load-phase DMA; the
seed statistics are computed from the bf16 image with partial sums
spread across DVE / ACT(accum) / GpSimd so they hide under the DMA.
Junk matmuls on otherwise-idle PE keep the HAM clock warm during load.

Sharding: pure data parallel, 4 samples per core on 8 cores.
"""
import sys

sys.path.insert(0, "/opt/trn_rl_repo")

from contextlib import ExitStack

import ml_dtypes
import numpy as np

import concourse.bacc as bacc
import concourse.bass as bass
import concourse.mybir as mybir
import concourse.tile as tile
from concourse.bass_utils import run_bass_kernel_spmd

B, CIN, COUT, KS, H, W = 32, 32, 32, 3, 192, 192
NCORES = 8
BC = B // NCORES          # 4 samples per core
P = BC * CIN              # 128 partitions = (sample, channel)
HP = H + 2                # 194 rows (one zero row above and below)
WP = W + 2                # 194: G gets the side padding instead of x
NPIX = HP * W             # 37248: row-padded pixels, rows contiguous
CHUNKS = [16] * 10 + [8] * 2 + [4] * 4   # input chunk rows (tapered tail)
NCHUNK = len(CHUNKS)      # 16
R = 2                     # output rows per conv tile
NT = H // R               # 96 conv tiles
NPAIR = NT // 2           # 48 stage1/evict pairs
NSLOT = 20                # G ring slots
LAG = 5                   # stage2 runs this many pairs behind stage1
RB = 2                    # stage1 pairs per replica-DMA batch
GT = 8                    # conv tiles per output DMA (16 rows)
NG = NT // GT             # 12 output DMAs
F32 = mybir.dt.float32
BF16 = mybir.dt.bfloat16
AX = mybir.AxisListType
OP = mybir.AluOpType
ACTF = mybir.ActivationFunctionType


def build_program(nc: bass.Bass) -> None:
    x_d = nc.dram_tensor("x", [BC, CIN, H, W], BF16, kind="ExternalInput").ap()
    wkh_d = nc.dram_tensor("wkh", [P, 81], F32, kind="ExternalInput").ap()
    lwin_d = nc.dram_tensor("lwin", [P, P], BF16, kind="ExternalInput").ap()
    brep_d = nc.dram_tensor("brep", [P, 1], F32, kind="ExternalInput").ap()
    wo9_d = nc.dram_tensor("wo9", [P, 9 * P], BF16, kind="ExternalInput").ap()
    m12_d = nc.dram_tensor("m12", [P, 12], F32, kind="ExternalInput").ap()
    ma_d = nc.dram_tensor("ma", [96, 1], F32, kind="ExternalInput").ap()
    ident_d = nc.dram_tensor("ident", [P, P], F32, kind="ExternalInput").ap()
    y_d = nc.dram_tensor("y", [BC, COUT, H, W], BF16, kind="ExternalOutput").ap()

    xf = x_d.rearrange("b c h w -> (b c) (h w)")       # [128, 36864] bf16
    yf = y_d.rearrange("b o h w -> (b o) (h w)")       # [128, 36864] bf16

    with tile.TileContext(nc) as tc, ExitStack() as ctx:
        const = ctx.enter_context(tc.tile_pool(name="const", bufs=1))
        ypool = ctx.enter_context(tc.tile_pool(name="ysb", bufs=3))
        psum_g = ctx.enter_context(tc.tile_pool(name="psum_g", bufs=2, space="PSUM"))
        psum_y = ctx.enter_context(tc.tile_pool(name="psum_y", bufs=2, space="PSUM"))

        xpad = const.tile([P, NPIX], BF16)
        wkh = const.tile([P, 81], F32)
        lwin = const.tile([P, P], BF16)
        brep = const.tile([P, 1], F32)
        wo9 = const.tile([P, 9 * P], BF16)
        m12 = const.tile([P, 12], F32)
        ma = const.tile([96, 1], F32)
        ident = const.tile([P, P], F32)
        scr = const.tile([P, 16 + 3 * NCHUNK], F32)  # 0:T 1:CF 2:CL 3:RF 4:RL 5..8 corners, then partials
        t81 = const.tile([P, 81], F32)
        kern = const.tile([P, 9], F32)
        kernb = const.tile([P, 9], BF16)
        vout = const.tile([P, 96], F32)
        lky = const.tile([P, 3 * P], BF16)     # stage1 lhsT per ky (cols (b,kx) live)
        lkx = const.tile([P, P], BF16)         # stage2 lhsT (kx=g in window g*32)
        gbuf = const.tile([P, NSLOT * R * WP], BF16)
        trash = const.tile([P, 16 * W], BF16)  # ACT accum-reduce dummy out

        x3 = xpad[:].rearrange("p (r c) -> p r c", c=W)    # [128, 194, 192]
        g4 = gbuf[:].rearrange("p (s r c) -> p s r c", s=NSLOT, c=WP)

        # constants ride the gpsimd (SWDGE) queue, parallel to the input;
        # issue them before the big gpsimd memsets so they are not delayed
        nc.gpsimd.dma_start(out=wkh[:], in_=wkh_d)
        nc.gpsimd.dma_start(out=lwin[:], in_=lwin_d)
        nc.gpsimd.dma_start(out=brep[:], in_=brep_d)
        nc.gpsimd.dma_start(out=wo9[:], in_=wo9_d)
        nc.gpsimd.dma_start(out=m12[:], in_=m12_d)
        nc.gpsimd.dma_start(out=ma[:], in_=ma_d)
        nc.gpsimd.dma_start(out=ident[:], in_=ident_d)

        # zero the two padding rows; column padding lives in the G buffer.
        # big memsets ride the otherwise-idle gpsimd engine
        nc.vector.memset(x3[:, 0, :], 0.0)
        nc.vector.memset(x3[:, HP - 1, :], 0.0)
        nc.gpsimd.memset(gbuf[:], 0.0)
        # lhsT tables are zero-padded to the full 128x128 array
        nc.gpsimd.memset(lky[:], 0.0)
        nc.gpsimd.memset(lkx[:], 0.0)
        nc.gpsimd.memset(vout[:], 0.0)

        # bf16 input lands straight in the padded image; partial sums for
        # the seed stats run on whichever engine has slack (T-sum touches
        # every element, so it is split DVE/ACT to hide under the DMA;
        # gpsimd has no free-axis reduce).  Tapered tail chunks keep the
        # last chunk's T-sum off the critical path.
        h0 = 0
        for i, lr in enumerate(CHUNKS):
            slot = x3[:, h0 + 1:h0 + 1 + lr, :]
            flat = xpad[:, (h0 + 1) * W:(h0 + 1 + lr) * W]
            nc.sync.dma_start(out=flat, in_=xf[:, h0 * W:(h0 + lr) * W])
            if i % 2 == 0:
                nc.vector.reduce_sum(out=scr[:, 16 + i:17 + i], in_=flat, axis=AX.X)
            else:
                nc.scalar.activation(
                    out=trash[:, 0:lr * W], in_=flat, func=ACTF.Copy,
                    accum_out=scr[:, 16 + i:17 + i])
            nc.vector.reduce_sum(
                out=scr[:, 16 + NCHUNK + i:17 + NCHUNK + i],
                in_=slot[:, :, 0], axis=AX.X)
            nc.vector.reduce_sum(
                out=scr[:, 16 + 2 * NCHUNK + i:17 + 2 * NCHUNK + i],
                in_=slot[:, :, W - 1], axis=AX.X)
            if i == 0:      # row-0 sum and top corners only need chunk 0
                nc.vector.reduce_sum(out=scr[:, 3:4], in_=slot[:, 0, :], axis=AX.X)
                nc.vector.tensor_copy(scr[:, 5:7], slot[:, 0, 0:W:W - 1])
            if i == NCHUNK - 1:  # last-row sum and bottom corners
                nc.vector.reduce_sum(out=scr[:, 4:5], in_=slot[:, lr - 1, :], axis=AX.X)
                nc.vector.tensor_copy(scr[:, 7:9], slot[:, lr - 1, 0:W:W - 1])
            # junk matmuls on late chunks keep the PE HAM clock warm so
            # stage1 opens at 2.4 GHz (PE is otherwise idle until then)
            if 8 <= i < 16:
                jnk = psum_y.tile([P, R * W], F32, tag="yps", name="jnk")
                nj = min(5, lr * W // 384)
                for w in range(nj):
                    nc.tensor.matmul(
                        jnk[:], lhsT=lwin[:],
                        rhs=xpad[:, (h0 + 1) * W + w * 384:(h0 + 1) * W + (w + 1) * 384],
                        start=(w == 0), stop=(w == nj - 1))
            h0 += lr

        # final sums: T/CF/CL in one grouped reduce
        nc.vector.reduce_sum(
            out=scr[:, 0:3],
            in_=scr[:, 16:16 + 3 * NCHUNK].rearrange("p (g i) -> p g i", g=3),
            axis=AX.X)

        # kernel[p, j] = sum_k wkH[p, j*9+k] * sums[p, k]
        sums9 = scr[:, 0:9].unsqueeze(1).broadcast_to([P, 9, 9])
        nc.vector.tensor_mul(t81[:].rearrange("p (j m) -> p j m", m=9), wkh[:].rearrange("p (j m) -> p j m", m=9), sums9)
        nc.vector.reduce_sum(
            out=kern[:], in_=t81[:].rearrange("p (j m) -> p j m", m=9), axis=AX.X)
        nc.vector.tensor_copy(kernb[:], kern[:])

        # win = blockdiag(w_in.T) @ kernel (+ b_in fused into the lky build)
        win_ps = psum_y.tile([P, 9], F32, tag="yps", name="win_ps")
        nc.tensor.matmul(win_ps[:], lhsT=lwin[:], rhs=kernb[:], start=True, stop=True)

        # stage1 weights: lky[(b,i), (b',kx)] = (win[b,i,3ky+kx]+b_in) d(b,b')
        m123 = m12[:].rearrange("p (b k) -> p b k", k=3)
        for ky in range(3):
            wv = win_ps[:, 3 * ky:3 * ky + 3].unsqueeze(1).broadcast_to([P, BC, 3])
            nc.vector.scalar_tensor_tensor(
                lky[:, ky * P:ky * P + 12].rearrange("p (b k) -> p b k", k=3),
                wv, brep[:], m123, op0=OP.add, op1=OP.mult)

        # wout / stage2-weight chain is deferred: stage1 only needs lky,
        # so it is issued after the first stage1 pairs (stage2 trails by
        # LAG pairs and the chain completes well before it starts)
        def build_stage2_weights():
            # wout[(b,o)] = sum_j blockdiag(w_out[:,:,j].T) @ kernel[:, j]
            wout_ps = psum_y.tile([P, 1], F32, tag="yps", name="wout_ps")
            for j in range(9):
                nc.tensor.matmul(
                    wout_ps[:], lhsT=wo9[:, j * P:(j + 1) * P],
                    rhs=kernb[:, j:j + 1], start=(j == 0), stop=(j == 8))
            # stage2 weights: w96[(32g+(b,kx)),(b',o)] = wout[b',o] d(b,b'),
            # then the kx mask keeps kx==g in window g (one matmul covers
            # all three taps since each window's G replica is col-shifted)
            for g in range(3):
                nc.vector.tensor_scalar_mul(
                    vout[:, 32 * g:32 * g + 12], m12[:], wout_ps[:, 0:1])
            w96_ps = psum_y.tile([96, P], F32, tag="yps", name="w96_ps")
            nc.tensor.transpose(w96_ps[:], vout[:], ident[:])
            nc.vector.tensor_scalar_mul(lkx[0:96, :], w96_ps[:], ma[:, 0:1])

        # conv: stage1 (G into PSUM 0:12), ACT evict at column offset 2,
        # two DMA replicas at offsets 1/0 into partition windows 32/64,
        # stage2 (one matmul covers all three kx taps), DVE y evict
        ysb_tiles = {}
        gps_cur = [None]
        yps_cur = [None]

        def stage1(t):
            h0 = t * R
            if t % 2 == 0:
                gps_cur[0] = psum_g.tile([P, 1024], F32, tag="gps", name="gps")
            out = gps_cur[0][:, (t % 2) * 512:(t % 2) * 512 + R * W]
            for ky in range(3):
                nc.tensor.matmul(
                    out,
                    lhsT=lky[:, ky * P:(ky + 1) * P],
                    rhs=x3[:, h0 + ky:h0 + ky + R, :],
                    start=(ky == 0), stop=(ky == 2))

        def evict_pair(p):
            # window g holds G at column offset 2-g so stage2's base-1
            # read of window g yields G[w+g-1] (kx = g).  The replicas are
            # FLAT shifted copies of the whole pair region: row-crossing
            # elements land in pad columns that are either never read or
            # copied from window0's always-zero columns 0/1.
            s = (2 * p) % NSLOT
            src = gps_cur[0][0:12, :].rearrange(
                "p (k b) -> p k b", k=2)[:, :, 0:R * W].rearrange(
                "p k (r w) -> p k r w", w=W)
            nc.scalar.copy(out=g4[0:12, s:s + 2, :, 2:2 + W], in_=src)
            # replica DMAs are batched RB pairs at a time (the DIRECT2D
            # issue cost is per dma_start); the drain pairs go singly so
            # the tail of the pipeline is not waiting on a batch boundary
            if p >= NPAIR - 6 or p % RB == RB - 1:
                rb = 1 if p >= NPAIR - 6 else RB
                base = ((2 * (p - rb + 1)) % NSLOT) * R * WP
                nb = 2 * rb * R * WP
                nc.scalar.dma_start(
                    out=gbuf[32:44, base:base + nb - 1],
                    in_=gbuf[0:12, base + 1:base + nb])
                nc.gpsimd.dma_start(
                    out=gbuf[64:76, base:base + nb - 2],
                    in_=gbuf[0:12, base + 2:base + nb])

        def stage2(t):
            g = t // GT
            if g not in ysb_tiles:
                ysb_tiles[g] = ypool.tile(
                    [P, GT * R * W], BF16, tag="ysb", name="ysb")
            ysb = ysb_tiles[g]
            if t % 2 == 0:
                yps_cur[0] = psum_y.tile([P, 1024], F32, tag="yps", name="yps")
            y_ps = yps_cur[0][:, (t % 2) * 512:(t % 2) * 512 + R * W]
            s = t % NSLOT
            nc.tensor.matmul(
                y_ps, lhsT=lkx[:], rhs=g4[:, s, :, 1:1 + W],
                start=True, stop=True)
            if t % 2 == 1:
                tt = t % GT
                ysrc = yps_cur[0][:].rearrange(
                    "p (k b) -> p k b", k=2)[:, :, 0:R * W]
                ydst = ysb[:, (tt - 1) * R * W:(tt + 1) * R * W].rearrange(
                    "p (k f) -> p k f", k=2)
                if t // 2 >= NPAIR - LAG and (t // 2) % 2 == 1:
                    nc.scalar.copy(out=ydst, in_=ysrc)
                else:
                    nc.vector.tensor_copy(ydst, ysrc)
                if g == NG - 1:
                    nc.sync.dma_start(
                        out=yf[:, (g * GT + tt - 1) * R * W:(g * GT + tt + 1) * R * W],
                        in_=ysb[:, (tt - 1) * R * W:(tt + 1) * R * W])
                elif tt == GT - 1:
                    nc.sync.dma_start(
                        out=yf[:, g * GT * R * W:(g + 1) * GT * R * W],
                        in_=ysb[:])
                    del ysb_tiles[g]

        # stage2 trails stage1 by LAG pairs so the PE queue never drains
        # waiting on the evict -> replica-DMA chain
        for p in range(NPAIR):
            stage1(2 * p)
            stage1(2 * p + 1)
            if p == 0:
                build_stage2_weights()
            if p >= LAG:
                stage2(2 * (p - LAG))
                stage2(2 * (p - LAG) + 1)
            evict_pair(p)
        for p in range(NPAIR - LAG, NPAIR):
            stage2(2 * p)
            stage2(2 * p + 1)


def host_tables(wk, w_in, b_in, w_out):
    # H matrix: sums vector [T,CF,CL,RF,RL,c00,c0L,cL0,cLL] -> S[m], m=(dy,dx)
    Hm = np.zeros((9, 9), np.float32)
    Hm[0, :] = 1.0
    for m in range(9):
        dy, dx = divmod(m, 3)
        if dy == 0:
            Hm[4, m] -= 1.0
        if dy == 2:
            Hm[3, m] -= 1.0
        if dx == 0:
            Hm[2, m] -= 1.0
        if dx == 2:
            Hm[1, m] -= 1.0
    Hm[8, 0] = Hm[7, 2] = Hm[6, 6] = Hm[5, 8] = 1.0
    wk9 = wk.reshape(CIN, 9, 9).astype(np.float32) / float(H * W)  # [c, j, m]
    wkh = np.einsum("cjm,km->cjk", wk9, Hm).reshape(CIN, 81)
    wkh = np.tile(wkh, (BC, 1))

    lwin = np.kron(np.eye(BC, dtype=np.float32), w_in.T.astype(np.float32))
    brep = np.tile(b_in.astype(np.float32), BC)[:, None]
    w9 = w_out.reshape(COUT, CIN, 9).astype(np.float32)
    wo9 = np.concatenate(
        [np.kron(np.eye(BC, dtype=np.float32), w9[:, :, j].T) for j in range(9)],
        axis=1)
    # m12[(b~,i), (b,kx)] = d(b~==b)
    m12 = np.repeat(np.eye(BC, dtype=np.float32), CIN, axis=0)
    m12 = np.repeat(m12, 3, axis=1)  # [128, 12]
    # row j of the stage2 table is (g, b, kx') with g=j//32, kx'=(j%32)%3
    # in the live windows [32g, 32g+12): keep kx'==g (window g's G replica
    # is column-shifted so its base-1 read is the kx=g tap)
    j = np.arange(96)
    live = (j % 32) < 12
    kxp = (j % 32) % 3
    ma = (live & (kxp == j // 32)).astype(np.float32)[:, None]
    ident = np.eye(P, dtype=np.float32)
    return {
        "wkh": np.ascontiguousarray(wkh, np.float32),
        "lwin": np.ascontiguousarray(lwin).astype(ml_dtypes.bfloat16),
        "brep": np.ascontiguousarray(brep, np.float32),
        "wo9": np.ascontiguousarray(wo9).astype(ml_dtypes.bfloat16),
        "m12": np.ascontiguousarray(m12, np.float32),
        "ma": np.ascontiguousarray(ma, np.float32),
        "ident": np.ascontiguousarray(ident, np.float32),
    }


_CACHE: dict = {}


def _get_program() -> bass.Bass:
    if "nc" not in _CACHE:
        nc = bacc.Bacc(
            trn_type="TRN2", target_bir_lowering=False, debug=False,
            num_devices=NCORES)
        build_program(nc)
        nc.compile()
        _CACHE["nc"] = nc
    return _CACHE["nc"]


def kernel(x, wk, w_in, b_in, w_out, _trace=False, _trace_kwargs=None):
    x = np.asarray(x).astype(ml_dtypes.bfloat16)
    tables = host_tables(np.asarray(wk), np.asarray(w_in), np.asarray(b_in),
                         np.asarray(w_out))
    nc = _get_program()
    in_maps = [
        {"x": np.ascontiguousarray(x[c * BC:(c + 1) * BC]), **tables}
        for c in range(NCORES)
    ]
    res = run_bass_kernel_spmd(
        nc, in_maps, core_ids=list(range(NCORES)),
        trace=_trace, **(_trace_kwargs or {}))
    y = np.concatenate(
        [np.asarray(res.results[c]["y"]).astype(np.float32)
         for c in range(NCORES)], axis=0)
    if _trace:
        return y, res
    return y


if __name__ == "__main__":
    rng = np.random.default_rng(0)
    inputs = {
        "x": rng.standard_normal((B, CIN, H, W), np.float32),
        "wk": rng.standard_normal((CIN * 9, 1, 3, 3)).astype(np.float32) * 0.05,
        "w_in": rng.standard_normal((CIN, CIN)).astype(np.float32) * 0.05,
        "b_in": rng.standard_normal((CIN,)).astype(np.float32) * 0.05,
        "w_out": rng.standard_normal((COUT, CIN, 3, 3)).astype(np.float32) * 0.05,
    }
    y = kernel(**inputs)
    print("y", y.shape, y.dtype, float(np.abs(y).max()))
